# revision 14
# baseline (speedup 1.0000x reference)
"""Trainium2 Bass kernel for nn_Decoder_60232621359478 (dense MoE decoder).

Model (per token): 3-layer gating MLP -> softmax over E=8 experts (dense
weights, all experts active), then 4 MoE layers where each layer is
  y = sum_e ew_e * ([z; x] @ W_e + b_e),  x <- elu(y) (except last layer).

Kernel strategy:
- Data-parallel over batch across 8 NeuronCores (B=32 -> 4 per core,
  1024 tokens/core). No collectives.
- Feature-major on chip (features on partitions, tokens on free axis).
- Expert gating folded into the matmul contraction:
    sum_e ew_e * (x @ W_e) = concat_e(ew_e * x) @ stack_e(W_e)
  Scaled inputs are produced by DVE + Pool (gpsimd) right before use.
- MoE layers 0-2 run in fp8 e4m3 with MatmulPerfMode.DoubleRow: each
  matmul contracts TWO 128-row k-tiles (stationary [128,2,128], moving
  [128,2,N]) at 0.5 cycles/row -- 2-4x the fp32r rate.  Scaling keeps
  everything in fp8 range: weights stored x64 (z-part) / x16 (x-part),
  z-inputs scaled x4, so PSUM = 256*y.  The carried activation is
  H = 256*(elu(y)+1) in bf16 (ACT: relu, exp(x+ln256); DVE/Pool: max+add),
  and the affine corrections fold into the next layer's bias.
- The final layer keeps its x-part in bf16 (fp8 there blows the error
  budget); its z-part reuses the shared fp8 z-tiles.
- The ew-scaled z pair-tiles (fp8) are built ONCE and shared by all 4
  MoE layers.
- v_hip (3 rows) + biases are folded into a single K=32 matmul per
  output tile: moving rows = [ew8(8); ew_e*v (3x8)] built via a PE
  broadcast, stationary = [256*b0; 256*Wv_e].
- Gating runs in bf16 (same PE rate as fp32r, half the SBUF/DMA).
- Elementwise scaling work is split across DVE and Pool (gpsimd).
"""

import math
import numpy as np
import ml_dtypes

import concourse.bass as bass
import concourse.mybir as mybir
import concourse.tile as tile
from concourse import bacc
from concourse import bass_utils

dt = mybir.dt
AF = mybir.ActivationFunctionType
ALU = mybir.AluOpType
PM = mybir.MatmulPerfMode

B, T = 32, 256
DM, DL, DH, DP, E = 256, 256, 512, 16, 8
NCORES = 8
BP = B // NCORES            # batches per core
NT = BP * T                 # tokens per core (1024)
CH = 2                      # token chunks
CT = NT // CH               # tokens per chunk (512)

# fp8 scaling scheme: PSUM holds 256*y for every MoE layer.
SZ = 4.0                    # mu for z / x0 inputs (alpha=1 sources)
WZ = 64.0                   # nu for z-part / x0-part weights
WX = 16.0                   # nu for x-part weights (layers 1..3)
MX = 1.0 / 16.0             # mu for x-part inputs (carried H = 256*h)
PS = 256.0                  # PSUM scale
LNP = math.log(PS)

E4 = ml_dtypes.float8_e4m3
BF = ml_dtypes.bfloat16

_CACHE = {}


def _prep_weights(gw0, gb0, gw1, gb1, gw2, gb2,
                  w0, b0, w1, b1, w2, b2, wo, bo):
    f = np.float32
    # gating (bf16): k-tiles [z0, z1, extra]; extra rows 0:16 = p, row 16 = b
    G0 = np.zeros((3, 128, DH), f)
    G0[0] = 0.25 * gw0[0:128]
    G0[1] = 0.25 * gw0[128:256]
    G0[2, 0:16] = gw0[256:272]
    G0[2, 16] = gb0

    def g_later(gw, gb, dout):
        Gt = np.zeros((7, 128, dout), f)
        Gt[0:6] = gw[0:768].reshape(6, 128, dout)
        Gt[0:2] *= 0.25                            # z arrives pre-scaled x4
        Gt[6, 16] = gb - gw[256:768].sum(axis=0)   # h' = elu+1 correction
        return Gt

    G1 = g_later(gw1, gb1, DH)
    G2 = g_later(gw2, gb2, E)

    # L0 fp8 pairs: [(z0,z1), (xc0,xc1)]; w0 rows are [z(256), v(3), xc(256)]
    W0P = np.zeros((E, 128, 3, 2, DH), f)
    w64 = w0 * WZ
    W0P[:, :, 0, 0] = w64[:, 0:128]
    W0P[:, :, 0, 1] = w64[:, 128:256]
    W0P[:, :, 1, 0] = w64[:, 259:387]
    W0P[:, :, 1, 1] = w64[:, 387:515]
    # L0 bias+v stationary [32, DH]: rows 0:8 = 256*b0; rows 8+3e.. = 256*Wv_e
    B0V = np.zeros((32, DH), f)
    B0V[0:8] = PS * b0
    for e in range(E):
        B0V[8 + 3 * e: 11 + 3 * e] = PS * w0[e, 256:259]

    def moe_mid(w, b):
        # pairs [(z0,z1)x64, (x0,x1)x64, (x2,x3)x64]; the carried
        # activation h64 = 4*elu(y) is centered, so no bias correction.
        Wt = np.zeros((E, 128, 3, 2, DH), f)
        Wt[:, :, 0, 0] = WZ * w[:, 0:128]
        Wt[:, :, 0, 1] = WZ * w[:, 128:256]
        Wt[:, :, 1, 0] = WZ * w[:, 256:384]
        Wt[:, :, 1, 1] = WZ * w[:, 384:512]
        Wt[:, :, 2, 0] = WZ * w[:, 512:640]
        Wt[:, :, 2, 1] = WZ * w[:, 640:768]
        return Wt, (PS * b).astype(f)

    W1P, B1 = moe_mid(w1, b1)
    W2P, B2 = moe_mid(w2, b2)

    # output layer: fully bf16 (fp8 noise on the low-gain output layer
    # dominates the error budget); rows [z(x64); x(x64)]
    WOB = np.ascontiguousarray(
        (WZ * wo[:, 0:768]).reshape(E, 6, 128, DM).transpose(0, 2, 1, 3))
    BO = (PS * bo).astype(f)

    ONES = np.ones((E, 128), f)
    EMAT = np.zeros((E, E * 128), f)
    for e in range(E):
        EMAT[e, e * 128:(e + 1) * 128] = 1.0
    # one-hot for the bias+v moving tile: row e -> col e and cols 8+3e..10+3e
    EVM = np.zeros((E, 32), f)
    for e in range(E):
        EVM[e, e] = 1.0
        EVM[e, 8 + 3 * e: 11 + 3 * e] = 1.0

    return dict(
        G0=G0.astype(BF), G1=G1.astype(BF), G2=G2.astype(BF),
        W0P=W0P.astype(E4), W1P=W1P.astype(E4), W2P=W2P.astype(E4),
        WOB=WOB.astype(BF),
        B0V=B0V, B1=B1, B2=B2, BO=BO,
        ONES=ONES, EMAT=EMAT, EVM=EVM)


def _prep_core_inputs(z, p_next, v_hip_next, x_curr, core):
    f = np.float32
    sl = slice(core * BP, (core + 1) * BP)
    zT = np.ascontiguousarray(SZ * z[sl].reshape(NT, DL).T.astype(f))
    x0T = np.ascontiguousarray(SZ * x_curr[sl].reshape(NT, DM).T.astype(f))
    gex = np.zeros((128, NT), f)
    gex[0:16] = p_next[sl].reshape(NT, DP).T
    gex[16] = 1.0
    # vz: rows 0:8 = 1 (identity for ew8), rows 8+3e.. = v (replicated)
    vz = np.zeros((32, NT), f)
    vz[0:8] = 1.0
    vT = v_hip_next[sl].reshape(NT, 3).T
    for e in range(E):
        vz[8 + 3 * e: 11 + 3 * e] = vT
    return dict(zT=zT.astype(BF), x0T=x0T.astype(BF), gex=gex.astype(BF),
                vz=vz.astype(BF))


def _build(repeat=1, scope="all"):
    nc = bacc.Bacc("TRN2", target_bir_lowering=False, debug=False,
                   num_devices=NCORES)
    # register ln(256) as a const AP so ACT can use it as an exp bias
    _lnp_t = nc.alloc_sbuf_tensor(f"const-float32-lnp", [128, 1], dt.float32)
    nc.gpsimd.memset(_lnp_t.ap(), LNP)
    nc.const_aps.aps[(dt.float32, LNP)] = _lnp_t.ap()
    _nf_t = nc.alloc_sbuf_tensor(f"const-float32-n4", [128, 1], dt.float32)
    nc.gpsimd.memset(_nf_t.ap(), -4.0)
    nc.const_aps.aps[(dt.float32, -4.0)] = _nf_t.ap()
    nc.all_engine_barrier()
    fr = dt.float32r
    f8 = dt.float8e4
    bf = dt.bfloat16
    f32 = dt.float32

    def din(name, shape, dtype):
        return nc.dram_tensor(name, shape, dtype, kind="ExternalInput").ap()

    zT_d = din("zT", (DL, NT), bf)
    x0T_d = din("x0T", (DM, NT), bf)
    gex_d = din("gex", (128, NT), bf)
    vz_d = din("vz", (32, NT), bf)
    ones_d = din("ONES", (E, 128), fr)
    emat_d = din("EMAT", (E, E * 128), fr)
    evm_d = din("EVM", (E, 32), fr)
    G0_d = din("G0", (3, 128, DH), bf)
    G1_d = din("G1", (7, 128, DH), bf)
    G2_d = din("G2", (7, 128, E), bf)
    W0_d = din("W0P", (E, 128, 3, 2, DH), f8)
    W1_d = din("W1P", (E, 128, 3, 2, DH), f8)
    W2_d = din("W2P", (E, 128, 3, 2, DH), f8)
    WOB_d = din("WOB", (E, 128, 6, DM), bf)
    B0V_d = din("B0V", (32, DH), fr)
    B1_d = din("B1", (E, DH), fr)
    B2_d = din("B2", (E, DH), fr)
    BO_d = din("BO", (E, DM), fr)
    yT_d = nc.dram_tensor("yT", (DM, NT), bf, kind="ExternalOutput").ap()

    with tile.TileContext(nc) as tc, \
         nc.allow_low_precision(reason="fp8/bf16 matmul quantization intended"):
        with tc.tile_pool(name="inp", bufs=1) as inp, \
             tc.tile_pool(name="wp", bufs=10) as wp, \
             tc.tile_pool(name="wo", bufs=1) as wop, \
             tc.tile_pool(name="act", bufs=1) as act, \
             tc.tile_pool(name="xsp", bufs=8) as xsp, \
             tc.tile_pool(name="xop", bufs=3) as xop, \
             tc.tile_pool(name="hp", bufs=3) as hp, \
             tc.tile_pool(name="tmp", bufs=2) as tmpp, \
             tc.tile_pool(name="ps", bufs=8, space="PSUM") as ps:

            # ---- persistent inputs ----
            z_sb = inp.tile([128, 2, NT], bf, name="z_sb")
            nc.sync.dma_start(z_sb, zT_d.rearrange("(k p) t -> p k t", p=128))
            x0_sb = inp.tile([128, 2, NT], bf, name="x0_sb")
            nc.sync.dma_start(x0_sb, x0T_d.rearrange("(k p) t -> p k t", p=128))
            gex_sb = inp.tile([128, NT], bf, name="gex_sb")
            nc.sync.dma_start(gex_sb, gex_d)
            vz_sb = inp.tile([32, NT], bf, name="vz_sb")
            nc.sync.dma_start(vz_sb, vz_d)
            ones_sb = inp.tile([E, 128], fr, name="ones_sb")
            nc.sync.dma_start(ones_sb, ones_d)
            emat_sb = inp.tile([E, E * 128], fr, name="emat_sb")
            nc.sync.dma_start(emat_sb, emat_d)
            evm_sb = inp.tile([E, 32], fr, name="evm_sb")
            nc.sync.dma_start(evm_sb, evm_d)
            g0_sb = inp.tile([128, 3, DH], bf, name="g0_sb")
            nc.sync.dma_start(g0_sb, G0_d.rearrange("k p d -> p k d"))
            g1_sb = inp.tile([128, 7, DH], bf, name="g1_sb")
            nc.sync.dma_start(g1_sb, G1_d.rearrange("k p d -> p k d"))
            g2_sb = inp.tile([128, 7, E], bf, name="g2_sb")
            nc.sync.dma_start(g2_sb, G2_d.rearrange("k p d -> p k d"))
            b0v_sb = inp.tile([32, DH], fr, name="b0v_sb")
            nc.sync.dma_start(b0v_sb, B0V_d)
            bias_sb = []
            for i, (bd, dout) in enumerate([(B1_d, DH), (B2_d, DH),
                                            (BO_d, DM)]):
                bt = inp.tile([E, dout], fr, name=f"b{i}_sb")
                nc.sync.dma_start(bt, bd)
                bias_sb.append(bt)

            def elu_p1(dst, psum):
                """dst = elu(psum) + 1 (gating, unscaled psum)."""
                mn = tmpp.tile([psum.shape[0], psum.shape[-1]], bf,
                               name="mn", tag="mn")
                nc.scalar.activation(mn[:, :], psum, AF.Relu, scale=-1.0)
                ex = tmpp.tile([psum.shape[0], psum.shape[-1]], bf,
                               name="ex", tag="ex")
                nc.scalar.activation(ex[:, :], mn[:, :], AF.Exp, scale=-1.0)
                nc.vector.scalar_tensor_tensor(
                    dst, psum, 0.0, ex[:, :], ALU.max, ALU.add)

            def elu256(dst, psum):
                """dst = 4*elu(psum/256)  (MoE, psum = 256*y; centered).

                A = max(psum,0) + 256*exp(min(psum/256,0));  dst = A/64 - 4.
                """
                mn = tmpp.tile([psum.shape[0], psum.shape[-1]], bf,
                               name="mn", tag="mn")
                nc.scalar.activation(mn[:, :], psum, AF.Relu, scale=-1.0 / PS)
                ex = tmpp.tile([psum.shape[0], psum.shape[-1]], bf,
                               name="ex", tag="ex")
                nc.scalar.activation(ex[:, :], mn[:, :], AF.Exp, scale=-1.0,
                                     bias=LNP)
                av = tmpp.tile([psum.shape[0], psum.shape[-1]], f32,
                               name="av", tag="av", bufs=2)
                nc.vector.scalar_tensor_tensor(
                    av[:, :], psum, 0.0, ex[:, :], ALU.max, ALU.add)
                nc.scalar.activation(dst, av[:, :], AF.Copy, scale=1.0 / 64.0,
                                     bias=-4.0)

            def body_gate():
                # ---- gating MLP (bf16) ----
                def glayer(w_sb, ktiles, rhs_of, douts, dst_of,
                           kt_order=None):
                    psums = [[ps.tile([128, CT], f32,
                                      name=f"gps{m}_{c}", tag="ps")
                              for c in range(CH)] for m in range(douts)]
                    order = list(kt_order) if kt_order else list(range(ktiles))
                    for kt in order:
                        for c in range(CH):
                            cs = slice(c * CT, (c + 1) * CT)
                            rhs = rhs_of(kt, cs)
                            for m in range(douts):
                                nc.tensor.matmul(
                                    psums[m][c][:, :] if douts > 1
                                    else psums[m][c][:E, :],
                                    w_sb[:, kt, m * 128:(m + 1) * 128]
                                    if douts > 1 else w_sb[:, kt, :],
                                    rhs,
                                    start=(kt == order[0]),
                                    stop=(kt == order[-1]))
                    for c in range(CH):
                        cs = slice(c * CT, (c + 1) * CT)
                        dst_of(c, cs, [psums[m][c] for m in range(douts)])

                h0 = [act.tile([128, NT], bf, name=f"h0_{m}", tag="xp",
                               bufs=8) for m in range(4)]

                def rhs_g0(kt, cs):
                    return (z_sb[:, kt, cs] if kt < 2 else gex_sb[:, cs])

                def dst_h0(c, cs, psums):
                    for m in range(4):
                        elu_p1(h0[m][:, cs], psums[m][:, :])

                glayer(g0_sb, 3, rhs_g0, 4, dst_h0)

                h1 = [act.tile([128, NT], bf, name=f"h1_{m}", tag="xp",
                               bufs=8) for m in range(4)]

                def rhs_g1(kt, cs):
                    if kt < 2:
                        return z_sb[:, kt, cs]
                    if kt < 6:
                        return h0[kt - 2][:, cs]
                    return gex_sb[:, cs]

                def dst_h1(c, cs, psums):
                    for m in range(4):
                        elu_p1(h1[m][:, cs], psums[m][:, :])

                glayer(g1_sb, 7, rhs_g1, 4, dst_h1,
                       kt_order=[0, 1, 6, 2, 3, 4, 5])

                exp_g = act.tile([E, NT], fr, name="exp_g", tag="eg")

                def rhs_g2(kt, cs):
                    if kt < 2:
                        return z_sb[:, kt, cs]
                    if kt < 6:
                        return h1[kt - 2][:, cs]
                    return gex_sb[:, cs]

                def dst_g2(c, cs, psums):
                    nc.scalar.activation(exp_g[:, cs], psums[0][:E, :], AF.Exp)

                glayer(g2_sb, 7, rhs_g2, 1, dst_g2,
                       kt_order=[0, 1, 6, 2, 3, 4, 5])

                # ---- softmax normalization + bias/v moving tile ----
                recip = act.tile([1, NT], fr, name="recip", tag="rc")
                mvt = act.tile([32, NT], fr, name="mvt", tag="mvt")
                for c in range(CH):
                    cs = slice(c * CT, (c + 1) * CT)
                    s_ps = ps.tile([1, CT], f32, name="s_ps", tag="ps")
                    nc.tensor.matmul(s_ps[:, :], ones_sb[:, 0:1],
                                     exp_g[:, cs], start=True, stop=True)
                    nc.vector.reciprocal(recip[:, cs], s_ps[:, :])
                    rb_ps = ps.tile([128, CT], f32, name="rb_ps", tag="ps")
                    nc.tensor.matmul(rb_ps[:, :], ones_sb[0:1, :],
                                     recip[:, cs], start=True, stop=True)
                    # ewall rows 0:8 = exp_g, rows 8+3e.. = exp_g[e] (bcast)
                    ev_ps = ps.tile([32, CT], f32, name="ev_ps", tag="ps")
                    nc.tensor.matmul(ev_ps[:, :], evm_sb[:, :], exp_g[:, cs],
                                     start=True, stop=True)
                    t32 = tmpp.tile([32, CT], f32, name="t32", tag="t32",
                                    bufs=2)
                    nc.vector.scalar_tensor_tensor(
                        t32[:, :], ev_ps[:, :], 1.0, vz_sb[:, cs],
                        ALU.mult, ALU.mult)
                    nc.vector.tensor_mul(mvt[:, cs], t32[:, :],
                                         rb_ps[0:32, :])

                # per-expert broadcast ew tiles (bf16)
                ewb = [act.tile([128, NT], bf, name=f"ewb{e}", tag="ewb",
                                bufs=8) for e in range(E)]
                for e in range(E):
                    for c in range(CH):
                        cs = slice(c * CT, (c + 1) * CT)
                        eb_ps = ps.tile([128, CT], f32,
                                        name="eb_ps", tag="ps")
                        nc.tensor.matmul(
                            eb_ps[:, :], emat_sb[:, e * 128:(e + 1) * 128],
                            mvt[0:8, cs], start=True, stop=True)
                        nc.scalar.copy(ewb[e][:, cs], eb_ps[:, :])

                # shared fp8 z pair-tiles: zp8[e][:,k,:] = 4*ew_e*z_k
                zp8 = [act.tile([128, 2, NT], f8, name=f"zp8_{e}", tag="zp8",
                                bufs=8) for e in range(E)]
                for e in range(E):
                    eng = nc.vector if e % 2 == 0 else nc.gpsimd
                    for k in range(2):
                        eng.tensor_mul(
                            zp8[e][:, k, :], z_sb[:, k, :], ewb[e][:, :])

                return mvt, ewb, zp8

            def body_moe(gate_out):
                mvt, ewb, zp8 = gate_out
                hcur = None

                for li in range(4):
                    douts = 2 if li == 3 else 4

                    # ---- weights (one rotating tag for the 3 fp8 layers) ----
                    if li < 3:
                        wd = (W0_d, W1_d, W2_d)[li]
                        wts = []
                        for e in range(E):
                            wt = wp.tile([128, 3, 2, DH], f8,
                                         name=f"w{li}_{e}", tag="wm", bufs=8)
                            nc.sync.dma_start(wt, wd[e])
                            wts.append(wt)
                    else:
                        wts = []
                        for e in range(E):
                            wb = wop.tile([128, 6, DM], bf, name=f"wob_{e}",
                                          tag="wob", bufs=8)
                            nc.sync.dma_start(wb, WOB_d[e])
                            wts.append(wb)

                    # ---- matmuls, c-major; xs tiles built per chunk ----
                    psums = [[ps.tile([128, CT], f32,
                                      name=f"mps{li}_{m}_{c}", tag="ps")
                              for c in range(CH)] for m in range(douts)]
                    for c in range(CH):
                        cs = slice(c * CT, (c + 1) * CT)
                        for m in range(douts):
                            ms = slice(m * 128, (m + 1) * 128)
                            if li == 0:
                                nc.tensor.matmul(
                                    psums[m][c][:, :], b0v_sb[:, ms],
                                    mvt[:, cs], start=True, stop=False)
                            else:
                                nc.tensor.matmul(
                                    psums[m][c][:, :], bias_sb[li - 1][:, ms],
                                    mvt[0:8, cs], start=True, stop=False)
                        for e in range(E):
                            last_e = (e == E - 1)
                            eng = nc.vector if (e + c) % 2 == 0 else nc.gpsimd
                            if li == 0:
                                xp = xsp.tile([128, 2, CT], f8,
                                              name=f"xs0_{e}_{c}",
                                              tag="xs", bufs=4)
                                for k in range(2):
                                    eng.tensor_mul(
                                        xp[:, k, :], x0_sb[:, k, cs],
                                        ewb[e][:, cs])
                                for m in range(douts):
                                    ms = slice(m * 128, (m + 1) * 128)
                                    nc.tensor.matmul(
                                        psums[m][c][:, :],
                                        wts[e][:, 0, :, ms], zp8[e][:, :, cs],
                                        perf_mode=PM.DoubleRow,
                                        start=False, stop=False)
                                    nc.tensor.matmul(
                                        psums[m][c][:, :],
                                        wts[e][:, 1, :, ms], xp[:, :, :],
                                        perf_mode=PM.DoubleRow,
                                        start=False, stop=last_e)
                            elif li < 3:
                                xp = xsp.tile([128, 2, 2, CT], f8,
                                              name=f"xs{li}_{e}_{c}",
                                              tag="xsm", bufs=6)
                                for pr in range(2):
                                    for k in range(2):
                                        eng.tensor_mul(
                                            xp[:, pr, k, :],
                                            hcur[:, 2 * pr + k, cs],
                                            ewb[e][:, cs])
                                for m in range(douts):
                                    ms = slice(m * 128, (m + 1) * 128)
                                    nc.tensor.matmul(
                                        psums[m][c][:, :],
                                        wts[e][:, 0, :, ms], zp8[e][:, :, cs],
                                        perf_mode=PM.DoubleRow,
                                        start=False, stop=False)
                                    for pr in range(2):
                                        nc.tensor.matmul(
                                            psums[m][c][:, :],
                                            wts[e][:, pr + 1, :, ms],
                                            xp[:, pr, :, :],
                                            perf_mode=PM.DoubleRow,
                                            start=False,
                                            stop=last_e and pr == 1)
                            else:
                                wb = wts[e]
                                xo = xop.tile([128, 6, CT], bf,
                                              name=f"xo_{e}_{c}",
                                              tag="xo", bufs=3)
                                for k in range(2):
                                    eng.tensor_mul(
                                        xo[:, k, :], z_sb[:, k, cs],
                                        ewb[e][:, cs])
                                for k in range(4):
                                    eng.tensor_mul(
                                        xo[:, 2 + k, :], hcur[:, k, cs],
                                        ewb[e][:, cs])
                                for m in range(douts):
                                    ms = slice(m * 128, (m + 1) * 128)
                                    for k in range(6):
                                        nc.tensor.matmul(
                                            psums[m][c][:, :],
                                            wb[:, k, ms], xo[:, k, :],
                                            start=False,
                                            stop=last_e and k == 5)

                    # ---- drain psums ----
                    if li < 3:
                        hnext = hp.tile([128, 4, NT], bf, name=f"H{li}",
                                        tag="H", bufs=2)
                        for c in range(CH):
                            cs = slice(c * CT, (c + 1) * CT)
                            for m in range(douts):
                                elu256(hnext[:, m, cs], psums[m][c][:, :])
                        hcur = hnext
                    else:
                        for c in range(CH):
                            cs = slice(c * CT, (c + 1) * CT)
                            for m in range(2):
                                yt = act.tile([128, CT], bf,
                                              name=f"y{m}_{c}", tag="y",
                                              bufs=2)
                                nc.scalar.activation(
                                    yt[:, :], psums[m][c][:, :],
                                    AF.Copy, scale=1.0 / PS)
                                nc.sync.dma_start(
                                    yT_d[m * 128:(m + 1) * 128, cs], yt)

            HINTS = (mybir.EngineType.PE, mybir.EngineType.DVE,
                     mybir.EngineType.Activation, mybir.EngineType.SP,
                     mybir.EngineType.Pool)
            if repeat == 1:
                body_moe(body_gate())
            elif scope == "all":
                with tc.For_i(0, repeat, 1, hint_engines=HINTS):
                    body_moe(body_gate())
            elif scope == "gating":
                with tc.For_i(0, repeat, 1, hint_engines=HINTS):
                    body_gate()
                body_moe(body_gate())
            elif scope == "moe":
                gout = body_gate()
                with tc.For_i(0, repeat, 1, hint_engines=HINTS):
                    body_moe(gout)
            else:
                raise ValueError(scope)

    nc.compile()
    return nc


class _Runner:
    """Held jitted 8-core runner (compile once, fast repeat dispatch)."""

    def __init__(self, nc, n_cores):
        import jax
        from jax.sharding import Mesh, PartitionSpec
        from jax.experimental.shard_map import shard_map
        from concourse.bass2jax import (_bass_exec_p, partition_id_tensor,
                                        install_neuronx_cc_hook,
                                        fast_dispatch_compile)
        install_neuronx_cc_hook()
        self.jax = jax
        self.PartitionSpec = PartitionSpec
        self.n_cores = n_cores
        partition_name = (nc.partition_id_tensor.name
                          if nc.partition_id_tensor else None)
        in_names, out_names, out_avals, zero_outs = [], [], [], []
        for alloc in nc.m.functions[0].allocations:
            if not isinstance(alloc, mybir.MemoryLocationSet):
                continue
            name = alloc.memorylocations[0].name
            if alloc.kind == "ExternalInput":
                if name != partition_name:
                    in_names.append(name)
            elif alloc.kind == "ExternalOutput":
                out_names.append(name)
                shape = tuple(alloc.tensor_shape)
                dtype = mybir.dt.np(alloc.dtype)
                out_avals.append(jax.core.ShapedArray(shape, dtype))
                zero_outs.append(np.zeros(shape, dtype))
        self.n_params = len(in_names)
        self.in_names = list(in_names)
        self.out_names = out_names
        self.out_avals = out_avals
        self.zero_outs = zero_outs
        all_in = list(in_names) + list(out_names)
        if partition_name is not None:
            all_in.append(partition_name)

        def _body(*args):
            operands = list(args)
            if partition_name is not None:
                operands.append(partition_id_tensor())
            outs = _bass_exec_p.bind(
                *operands,
                out_avals=tuple(out_avals),
                in_names=tuple(all_in),
                out_names=tuple(out_names),
                lowering_input_output_aliases=(),
                sim_require_finite=True,
                sim_require_nnan=True,
                nc=nc,
            )
            return tuple(outs)

        devices = jax.devices()[:n_cores]
        self.mesh = Mesh(np.asarray(devices), ("core",))
        self._n_in = self.n_params + len(out_names)
        self._body = _body
        self._shard_map = shard_map
        self._fd_compile = fast_dispatch_compile
        self.fn = None

    def _compile(self, dev):
        jax, P = self.jax, self.PartitionSpec

        def compile_fn():
            jitted = jax.jit(
                self._shard_map(self._body, mesh=self.mesh,
                                in_specs=(P("core"),) * self._n_in,
                                out_specs=(P("core"),) * len(self.out_names),
                                check_rep=False),
                keep_unused=True)
            return jitted.lower(*dev).compile()
        self.fn = self._fd_compile(compile_fn)

    def run(self, in_maps):
        jax, P = self.jax, self.PartitionSpec
        n = self.n_cores
        per_core = [[np.asarray(m[name]) for name in self.in_names]
                    for m in in_maps]
        concat_in = [np.concatenate([per_core[c][i] for c in range(n)],
                                    axis=0) for i in range(self.n_params)]
        concat_zeros = [np.zeros((n * z.shape[0], *z.shape[1:]), z.dtype)
                        for z in self.zero_outs]
        sh = jax.sharding.NamedSharding(self.mesh, P("core"))
        dev = [jax.device_put(a, sh) for a in concat_in + concat_zeros]
        for d in dev:
            d.block_until_ready()
        if self.fn is None:
            self._compile(dev)
        outs = [np.asarray(o) for o in self.fn(*dev)]
        results = [
            {name: outs[i].reshape(n, *self.out_avals[i].shape)[c]
             for i, name in enumerate(self.out_names)}
            for c in range(n)
        ]
        return results, dev


def kernel(z, p_next, v_hip_next, x_curr,
           gw0, gb0, gw1, gb1, gw2, gb2,
           w0, b0, w1, b1, w2, b2, wo, bo):
    if "nc" not in _CACHE:
        _CACHE["nc"] = _build()
        _CACHE["runner"] = _Runner(_CACHE["nc"], NCORES)

    wdict = _prep_weights(
        np.asarray(gw0, np.float32), np.asarray(gb0, np.float32),
        np.asarray(gw1, np.float32), np.asarray(gb1, np.float32),
        np.asarray(gw2, np.float32), np.asarray(gb2, np.float32),
        np.asarray(w0, np.float32), np.asarray(b0, np.float32),
        np.asarray(w1, np.float32), np.asarray(b1, np.float32),
        np.asarray(w2, np.float32), np.asarray(b2, np.float32),
        np.asarray(wo, np.float32), np.asarray(bo, np.float32))

    in_maps = []
    for c in range(NCORES):
        m = _prep_core_inputs(np.asarray(z, np.float32),
                              np.asarray(p_next, np.float32),
                              np.asarray(v_hip_next, np.float32),
                              np.asarray(x_curr, np.float32), c)
        m.update(wdict)
        in_maps.append(m)

    results, _ = _CACHE["runner"].run(in_maps)

    out = np.empty((B, T, DM), np.float32)
    for c in range(NCORES):
        yT = np.asarray(results[c]["yT"], np.float32)   # (DM, NT)
        out[c * BP:(c + 1) * BP] = yT.T.reshape(BP, T, DM)
    return out
